# revision 22
# baseline (speedup 1.0000x reference)
"""BitLinear (RMSNorm + ternary-quantized linear) Trainium2 kernel.

Full-input contract: kernel(**inputs) takes the unsharded numpy inputs and
returns the full [B, S, DOUT] float32 output.

Strategy (column parallel over 8 NeuronCores):
  - Host: thr = mean(|w|) (computed with CPU jax to match the fp32 reduction
    order of the reference), ternarize w -> {-1,0,+1}, fold gamma in, cast
    to fp8 e4m3 (exact for ternary values), lay out as DoubleRow k-pairs.
    x is split hi = e4m3(x), lo = e4m3(x - hi); hi covers all 2048
    features, lo covers the first LO_KP*256 = 1024 features (measured
    rel err 1.7e-2 vs the 2e-2 budget; full-lo would cost as much as
    bf16 and hi-only measures 2.7e-2).
  - Device (per core, SPMD): weight shard [128, KP, 2, 1024] e4m3 resident
    in SBUF. Stream x in chunks of 512 tokens. Main matmuls run in fp8
    DoubleRow perf mode (256-feature contraction per pass, 2x bf16 FLOPS):
    per 128-token group, 8 hi + LO_KP lo matmuls accumulate into two
    [128, 512] PSUM tiles over k-pairs.
    RMSNorm: sum(x^2) per token from hi via one ScalarE Square, a 4-deep
    tree-add over the k axis on the Pool engine (GPSIMD) so the DVE stays
    off the critical path, and a tiny bf16 ones-matmul for the
    cross-partition reduction; r = 1/sqrt(sum/DIN + eps) via ScalarE Sqrt
    + VectorE reciprocal (ScalarE Rsqrt is banned for accuracy). PSUM is
    evicted with a VectorE tensor_scalar_mul by r.
  - Host gathers the 8 [M, 1024] shards along the feature axis and adds
    bias there (exact fp32; it is all-zeros for this problem).
The bf16 predecessor of this kernel ran at the bf16 PE roofline (~1.05 ms
claimed; 1.18 ms re-measured under current device conditions). fp8
DoubleRow halves the PE stream time of the hi pass, and the half-coverage
lo pass costs 0.25x, for ~0.75x total PE work plus a halved per-matmul
fixed overhead (~35 ns/MM: EVT_SEM write + decode; walrus asserts
UpdateValue==1 so sem increments cannot be batched). Measured on the 8
axon trn2 cores: 796-942 us per-core HW exec across repeated trials
(same-methodology bf16 baseline: 1184 us), i.e. ~1.35x. Error on the
real inputs (deterministic): max|err|/max|ref| = 1.69e-2,
||err||/||ref|| = 1.88e-2, mean|err|/mean|ref| = 1.88e-2 — all under
the 2e-2 gate; the norm pipeline (ScalarE square -> Pool tree-add ->
ones-matmul -> Sqrt -> reciprocal) is fully hidden behind the PE stream,
with the ones-matmuls placed after the first main matmul group so a
chunk boundary never stalls the PE on the reduction chain.
"""

import numpy as np

B, S, DIN, DOUT = 4, 4096, 2048, 8192
M = B * S  # 16384
NCORES = 8
NSHARD = DOUT // NCORES  # 1024
P = 128
KT = DIN // P  # 16 k-tiles
KP = KT // 2  # 8 DoubleRow k-pairs
MCH = 512  # tokens per chunk
G = MCH // P  # 4 groups of 128 tokens per chunk
H = NSHARD // 512  # 2 n-halves
LO_KP = 4  # k-pairs (256 features each) that get the lo-residual pass
EPS = float(np.finfo(np.float32).eps)

_CACHE = {}


def build_nc(m_tokens=M, lo_kp=LO_KP, reps=1, psum_wide=False,
             ones_at_head=False):
    # Closed optimization avenues, so they are not retried:
    #  - psum_wide=True (one [128,1024] 2-bank PSUM output per matmul,
    #    halving matmul count): walrus rejects it, ISA caps matmul
    #    moving/output elements at 512 ('s3d3_mm_num_elements').
    #  - Batching per-matmul PE sem increments onto the last matmul of a
    #    wait-free run (saves ~26ns/MM): walrus codegen asserts
    #    `UpdateValue == 1` for EVT_SEM updates, so batched increments
    #    cannot be encoded in a NEFF at all.
    #  - Strided DVE tensor_reduce for the k-axis reduction (innermost
    #    stride 2KB): measured ~150us slower than the Pool tree-add.
    #  - xin bufs=3 deep prefetch: measured ~195us slower than bufs=2.
    import contextlib

    import concourse.bacc as bacc
    import concourse.mybir as mybir
    import concourse.tile as tile

    nch = m_tokens // MCH
    f32 = mybir.dt.float32
    bf16 = mybir.dt.bfloat16
    fp8 = mybir.dt.float8e4
    DR = mybir.MatmulPerfMode.DoubleRow
    Sqrt = mybir.ActivationFunctionType.Sqrt

    nc = bacc.Bacc("TRN2", target_bir_lowering=False, debug=False,
                   num_devices=NCORES)
    # packed hi: xhi[c*P+p, kp, i, m] = e4m3(x)[c*MCH+m, kp*256+i*128+p]
    xhi_h = nc.dram_tensor("xhi", [nch * P, KP, 2, MCH], fp8,
                           kind="ExternalInput")
    xhi = xhi_h.ap()
    if lo_kp:
        xlo_h = nc.dram_tensor("xlo", [nch * P, lo_kp, 2, MCH], fp8,
                               kind="ExternalInput")
        xlo = xlo_h.ap()
    # wt[p, kp, i, n] = weff.T[kp*256+i*128+p, n]
    wt_h = nc.dram_tensor("wt", [P, KP, 2, NSHARD], fp8,
                          kind="ExternalInput")
    wt = wt_h.ap()
    out_h = nc.dram_tensor("out", [m_tokens, NSHARD], f32,
                           kind="ExternalOutput")
    out = out_h.ap()

    with tile.TileContext(nc) as tc:
        with (
            tc.tile_pool(name="const", bufs=1) as constp,
            tc.tile_pool(name="xin", bufs=2) as xin,
            tc.tile_pool(name="sq", bufs=3) as sqp,
            tc.tile_pool(name="nrm", bufs=2) as nrmp,
            tc.tile_pool(name="ev", bufs=4) as evp,
            tc.tile_pool(name="ps", bufs=(3 if psum_wide else 5),
                         space="PSUM") as psp,
            tc.tile_pool(name="psms", bufs=2, space="PSUM") as psmsp,
        ):
            w_sb = constp.tile([P, KP, 2, NSHARD], fp8)
            nc.sync.dma_start(w_sb[:], wt[:])
            ones_col = constp.tile([P, 1], bf16)
            nc.vector.memset(ones_col[:], 1.0)
            eps_col = constp.tile([P, 1], f32)
            nc.vector.memset(eps_col[:], EPS)

            rep_ctx = (tc.For_i(0, reps, 1) if reps > 1
                       else contextlib.nullcontext())
            with rep_ctx:
              for c in range(nch):
                m0 = c * MCH
                x_hi = xin.tile([P, KP, 2, MCH], fp8, tag="xhi")
                nc.sync.dma_start(x_hi[:], xhi[c * P:(c + 1) * P])
                if lo_kp:
                    x_lo = xin.tile([P, lo_kp, 2, MCH], fp8, tag="xlo")
                    nc.sync.dma_start(x_lo[:], xlo[c * P:(c + 1) * P])

                # --- RMSNorm: sum over features of hi^2, spread across
                # partitions (p) and the free k axis. Square on ScalarE,
                # k-axis tree-add on Pool (keeps DVE free so the next
                # chunk's reduction overlaps this chunk's matmuls), then a
                # bf16 ones-matmul for the cross-partition sum.
                sqf = sqp.tile([P, KT * MCH], mybir.dt.float32, tag="sqf")
                nc.scalar.square(sqf[:],
                                 x_hi[:].rearrange("p k i m -> p (k i m)"))
                half = KT * MCH // 2
                while half >= MCH:
                    nc.gpsimd.tensor_add(sqf[:, :half], sqf[:, :half],
                                         sqf[:, half:2 * half])
                    half //= 2
                acc_bf = sqp.tile([P, MCH], bf16, tag="accbf")
                nc.gpsimd.tensor_copy(acc_bf[:], sqf[:, :MCH])

                # --- main fp8 DoubleRow matmuls ---
                # psum_wide: one [128, 1024] PSUM tile spanning 2 banks per
                # token group (halves the matmul count and its ~70ns/MM
                # fixed overhead); otherwise two [128, 512] single-bank
                # tiles.
                def emit_norm_tail():
                    ps_ms = psmsp.tile([P, G], mybir.dt.float32, tag="ms")
                    for gg in range(G):
                        nc.tensor.matmul(
                            ps_ms[:, gg:gg + 1],
                            acc_bf[:, gg * P:(gg + 1) * P],
                            ones_col[:], start=True, stop=True)
                    # r = 1 / sqrt(sum/DIN + eps)
                    sqms = nrmp.tile([P, G], mybir.dt.float32, tag="sqms")
                    nc.scalar.activation(sqms[:], ps_ms[:], Sqrt,
                                         bias=eps_col[:], scale=1.0 / DIN)
                    r = nrmp.tile([P, G], mybir.dt.float32, tag="r")
                    nc.vector.reciprocal(r[:], sqms[:])
                    return r

                if ones_at_head:
                    r_sb = emit_norm_tail()
                NW = NSHARD if psum_wide else 512
                HW_ = NSHARD // NW
                for g in range(G):
                    pss = [psp.tile([P, NW], mybir.dt.float32,
                                    tag="ps", name=f"ps{c}_{g}_{h}")
                           for h in range(HW_)]
                    nmm = KP + lo_kp
                    i = 0
                    for kp in range(KP):
                        for h in range(HW_):
                            nc.tensor.matmul(
                                pss[h][:],
                                x_hi[:, kp, :, g * P:(g + 1) * P],
                                w_sb[:, kp, :, h * NW:(h + 1) * NW],
                                start=(i == 0), stop=(i == nmm - 1),
                                perf_mode=DR)
                        i += 1
                    for kp in range(lo_kp):
                        for h in range(HW_):
                            nc.tensor.matmul(
                                pss[h][:],
                                x_lo[:, kp, :, g * P:(g + 1) * P],
                                w_sb[:, kp, :, h * NW:(h + 1) * NW],
                                start=(i == 0), stop=(i == nmm - 1),
                                perf_mode=DR)
                        i += 1
                    if g == 0 and not ones_at_head:
                        # ones-matmuls sit after the first main group so
                        # the PE never stalls on the reduction chain at a
                        # chunk boundary; r is still ready before the
                        # first eviction needs it.
                        r_sb = emit_norm_tail()
                    for h in range(HW_):
                        ev = evp.tile([P, NW], mybir.dt.float32, tag="ev")
                        nc.vector.tensor_scalar_mul(
                            ev[:], pss[h][:], r_sb[:, g:g + 1])
                        nc.sync.dma_start(
                            out[m0 + g * P:m0 + (g + 1) * P,
                                h * NW:(h + 1) * NW],
                            ev[:])
    nc.compile()
    return nc


def _host_prep(x, weight, bias, gamma, lo_kp=LO_KP):
    import jax
    import jax.numpy as jnp
    import ml_dtypes

    e4 = ml_dtypes.float8_e4m3
    w32 = np.asarray(weight, np.float32)
    try:
        # CPU jax reproduces the reference's fp32 reduction order bitwise;
        # ~2 weights sit within 1 ulp of thr, so the order matters.
        with jax.default_device(jax.devices("cpu")[0]):
            thr = np.float32(jnp.mean(jnp.abs(jnp.asarray(w32))))
    except Exception:
        thr = np.float32(np.mean(np.abs(w32)))
    wq = (np.sign(w32) * (np.abs(w32) > thr)).astype(np.float32)
    weff = wq * np.asarray(gamma, np.float32)[None, :]  # [DOUT, DIN]
    # w8[p, kp, i, n] = weff.T[kp*256 + i*128 + p, n]
    # (exact in e4m3 for ternary weights with gamma == 1)
    w8 = np.ascontiguousarray(
        weff.T.reshape(KP, 2, P, DOUT).transpose(2, 0, 1, 3)
    ).astype(e4)  # [P, KP, 2, DOUT]

    x32 = np.asarray(x, np.float32).reshape(M, DIN)
    hi = x32.astype(e4)
    lo = (x32 - hi.astype(np.float32)).astype(e4)

    def pack(a, kp_lim):
        # a[m, f]; f = kp*256 + i*128 + p -> [nch*P, kp, i, MCH]
        return np.ascontiguousarray(
            a.reshape(M // MCH, MCH, KP, 2, P)[:, :, :kp_lim]
            .transpose(0, 4, 2, 3, 1)
        ).reshape((M // MCH) * P, kp_lim, 2, MCH)

    b32 = np.ascontiguousarray(np.asarray(bias, np.float32))
    return pack(hi, KP), (pack(lo, lo_kp) if lo_kp else None), w8, b32


def _in_maps(xhi, xlo, w8):
    maps = []
    for c in range(NCORES):
        m = {"xhi": xhi,
             "wt": np.ascontiguousarray(
                 w8[:, :, :, c * NSHARD:(c + 1) * NSHARD])}
        if xlo is not None:
            m["xlo"] = xlo
        maps.append(m)
    return maps


def kernel(x, weight, bias, gamma):
    from concourse.bass_utils import run_bass_kernel_spmd

    if "nc" not in _CACHE:
        _CACHE["nc"] = build_nc()
    nc = _CACHE["nc"]

    xhi, xlo, w8, b32 = _host_prep(x, weight, bias, gamma)
    in_maps = _in_maps(xhi, xlo, w8)
    res = run_bass_kernel_spmd(nc, in_maps, core_ids=list(range(NCORES)))
    shards = [res.results[c]["out"] for c in range(NCORES)]
    full = np.concatenate(shards, axis=1)
    if np.any(b32):
        full += b32[None, :]
    return np.ascontiguousarray(
        full.reshape(B, S, DOUT).astype(np.float32, copy=False))


# revision 26
# speedup vs baseline: 1.1917x; 1.1917x over previous
"""BitLinear (RMSNorm + ternary-quantized linear) Trainium2 kernel.

Full-input contract: kernel(**inputs) takes the unsharded numpy inputs and
returns the full [B, S, DOUT] float32 output.

Strategy (column parallel over 8 NeuronCores):
  - Host: thr = mean(|w|) (computed with CPU jax to match the fp32 reduction
    order of the reference), ternarize w -> {-1,0,+1}, fold gamma in, cast
    to fp8 e4m3 (exact for ternary values), lay out as DoubleRow k-pairs.
    x is split hi = e4m3(x), lo = e4m3(x - hi); hi covers all 2048
    features, lo covers the first LO_KP*256 = 1024 features (measured
    rel err 1.7e-2 vs the 2e-2 budget; full-lo would cost as much as
    bf16 and hi-only measures 2.7e-2).
  - Device (per core, SPMD): weight shard [128, KP, 2, 1024] e4m3 resident
    in SBUF. Stream x in chunks of 512 tokens. Main matmuls run in fp8
    DoubleRow perf mode (256-feature contraction per pass, 2x bf16 FLOPS):
    per 128-token group, 8 hi + LO_KP lo matmuls accumulate into two
    [128, 512] PSUM tiles over k-pairs.
    RMSNorm: sum(x^2) per token from hi via one ScalarE Square, a 4-deep
    tree-add over the k axis on the Pool engine (GPSIMD) so the DVE stays
    off the critical path, and a tiny bf16 ones-matmul for the
    cross-partition reduction; r = 1/sqrt(sum/DIN + eps) via ScalarE Sqrt
    + VectorE reciprocal (ScalarE Rsqrt is banned for accuracy). PSUM is
    evicted with a VectorE tensor_scalar_mul by r.
  - Host gathers the 8 [M, 1024] shards along the feature axis and adds
    bias there (exact fp32; it is all-zeros for this problem).
The bf16 predecessor of this kernel ran at the bf16 PE roofline (~1.05 ms
claimed; 1.18 ms re-measured under current device conditions). fp8
DoubleRow halves the PE stream time of the hi pass, and the half-coverage
lo pass costs 0.25x, for ~0.75x total PE work plus a halved per-matmul
fixed overhead (~35 ns/MM: EVT_SEM write + decode; walrus asserts
UpdateValue==1 so sem increments cannot be batched). Measured on the 8
axon trn2 cores: 796-942 us per-core HW exec across repeated trials
(same-methodology bf16 baseline: 1184 us), i.e. ~1.35x. Error on the
real inputs (deterministic): max|err|/max|ref| = 1.69e-2,
||err||/||ref|| = 1.88e-2, mean|err|/mean|ref| = 1.88e-2 — all under
the 2e-2 gate; the norm pipeline (ScalarE square -> Pool tree-add ->
ones-matmul -> Sqrt -> reciprocal) is fully hidden behind the PE stream,
with the ones-matmuls placed after the first main matmul group so a
chunk boundary never stalls the PE on the reduction chain.
"""

import numpy as np

B, S, DIN, DOUT = 4, 4096, 2048, 8192
M = B * S  # 16384
NCORES = 8
NSHARD = DOUT // NCORES  # 1024
P = 128
KT = DIN // P  # 16 k-tiles
KP = KT // 2  # 8 DoubleRow k-pairs
MCH = 512  # tokens per chunk
G = MCH // P  # 4 groups of 128 tokens per chunk
H = NSHARD // 512  # 2 n-halves
LO_KP = 4  # k-pairs (256 features each) that get the lo-residual pass
EPS = float(np.finfo(np.float32).eps)

_CACHE = {}


def build_nc(m_tokens=M, lo_kp=LO_KP, reps=1, psum_wide=False,
             ones_at_head=False, out_bf16=False):
    # Closed optimization avenues, so they are not retried:
    #  - psum_wide=True (one [128,1024] 2-bank PSUM output per matmul,
    #    halving matmul count): walrus rejects it, ISA caps matmul
    #    moving/output elements at 512 ('s3d3_mm_num_elements').
    #  - Batching per-matmul PE sem increments onto the last matmul of a
    #    wait-free run (saves ~26ns/MM): walrus codegen asserts
    #    `UpdateValue == 1` for EVT_SEM updates, so batched increments
    #    cannot be encoded in a NEFF at all.
    #  - Strided DVE tensor_reduce for the k-axis reduction (innermost
    #    stride 2KB): measured ~150us slower than the Pool tree-add.
    #  - xin bufs=3 deep prefetch: measured ~195us slower than bufs=2.
    #  - out_bf16=True (halves out traffic + DVE write bytes): window-
    #    paired A/B showed no win beyond device noise (means 963 vs 934
    #    us across 3 trials with one reversal) and it costs accuracy
    #    margin (1.716e-2 vs 1.692e-2); PE is the bottleneck and the out
    #    DMA had ~2x slack, so f32 out is kept.
    #  - lo on 3 k-pairs (768 features) instead of 4: 2.10e-2, over the
    #    2e-2 gate; coverage granularity is one 256-feature k-pair per
    #    512-cycle matmul, so 4 pairs is the minimum passing point.
    import contextlib

    import concourse.bacc as bacc
    import concourse.mybir as mybir
    import concourse.tile as tile

    nch = m_tokens // MCH
    f32 = mybir.dt.float32
    bf16 = mybir.dt.bfloat16
    fp8 = mybir.dt.float8e4
    DR = mybir.MatmulPerfMode.DoubleRow
    Sqrt = mybir.ActivationFunctionType.Sqrt

    nc = bacc.Bacc("TRN2", target_bir_lowering=False, debug=False,
                   num_devices=NCORES)
    # packed hi: xhi[c*P+p, kp, i, m] = e4m3(x)[c*MCH+m, kp*256+i*128+p]
    xhi_h = nc.dram_tensor("xhi", [nch * P, KP, 2, MCH], fp8,
                           kind="ExternalInput")
    xhi = xhi_h.ap()
    if lo_kp:
        xlo_h = nc.dram_tensor("xlo", [nch * P, lo_kp, 2, MCH], fp8,
                               kind="ExternalInput")
        xlo = xlo_h.ap()
    # wt[p, kp, i, n] = weff.T[kp*256+i*128+p, n]
    wt_h = nc.dram_tensor("wt", [P, KP, 2, NSHARD], fp8,
                          kind="ExternalInput")
    wt = wt_h.ap()
    out_h = nc.dram_tensor("out", [m_tokens, NSHARD],
                           bf16 if out_bf16 else f32,
                           kind="ExternalOutput")
    out = out_h.ap()

    with tile.TileContext(nc) as tc:
        with (
            tc.tile_pool(name="const", bufs=1) as constp,
            tc.tile_pool(name="xin", bufs=2) as xin,
            tc.tile_pool(name="sq", bufs=3) as sqp,
            tc.tile_pool(name="nrm", bufs=2) as nrmp,
            tc.tile_pool(name="ev", bufs=4) as evp,
            tc.tile_pool(name="ps", bufs=(3 if psum_wide else 5),
                         space="PSUM") as psp,
            tc.tile_pool(name="psms", bufs=2, space="PSUM") as psmsp,
        ):
            w_sb = constp.tile([P, KP, 2, NSHARD], fp8)
            nc.sync.dma_start(w_sb[:], wt[:])
            ones_col = constp.tile([P, 1], bf16)
            nc.vector.memset(ones_col[:], 1.0)
            eps_col = constp.tile([P, 1], f32)
            nc.vector.memset(eps_col[:], EPS)

            rep_ctx = (tc.For_i(0, reps, 1) if reps > 1
                       else contextlib.nullcontext())
            with rep_ctx:
              for c in range(nch):
                m0 = c * MCH
                x_hi = xin.tile([P, KP, 2, MCH], fp8, tag="xhi")
                nc.sync.dma_start(x_hi[:], xhi[c * P:(c + 1) * P])
                if lo_kp:
                    x_lo = xin.tile([P, lo_kp, 2, MCH], fp8, tag="xlo")
                    nc.sync.dma_start(x_lo[:], xlo[c * P:(c + 1) * P])

                # --- RMSNorm: sum over features of hi^2, spread across
                # partitions (p) and the free k axis. Square on ScalarE,
                # k-axis tree-add on Pool (keeps DVE free so the next
                # chunk's reduction overlaps this chunk's matmuls), then a
                # bf16 ones-matmul for the cross-partition sum.
                sqf = sqp.tile([P, KT * MCH], mybir.dt.float32, tag="sqf")
                nc.scalar.square(sqf[:],
                                 x_hi[:].rearrange("p k i m -> p (k i m)"))
                half = KT * MCH // 2
                while half >= MCH:
                    nc.gpsimd.tensor_add(sqf[:, :half], sqf[:, :half],
                                         sqf[:, half:2 * half])
                    half //= 2
                acc_bf = sqp.tile([P, MCH], bf16, tag="accbf")
                nc.gpsimd.tensor_copy(acc_bf[:], sqf[:, :MCH])

                # --- main fp8 DoubleRow matmuls ---
                # psum_wide: one [128, 1024] PSUM tile spanning 2 banks per
                # token group (halves the matmul count and its ~70ns/MM
                # fixed overhead); otherwise two [128, 512] single-bank
                # tiles.
                def emit_norm_tail():
                    ps_ms = psmsp.tile([P, G], mybir.dt.float32, tag="ms")
                    for gg in range(G):
                        nc.tensor.matmul(
                            ps_ms[:, gg:gg + 1],
                            acc_bf[:, gg * P:(gg + 1) * P],
                            ones_col[:], start=True, stop=True)
                    # r = 1 / sqrt(sum/DIN + eps)
                    sqms = nrmp.tile([P, G], mybir.dt.float32, tag="sqms")
                    nc.scalar.activation(sqms[:], ps_ms[:], Sqrt,
                                         bias=eps_col[:], scale=1.0 / DIN)
                    r = nrmp.tile([P, G], mybir.dt.float32, tag="r")
                    nc.vector.reciprocal(r[:], sqms[:])
                    return r

                if ones_at_head:
                    r_sb = emit_norm_tail()
                NW = NSHARD if psum_wide else 512
                HW_ = NSHARD // NW
                for g in range(G):
                    pss = [psp.tile([P, NW], mybir.dt.float32,
                                    tag="ps", name=f"ps{c}_{g}_{h}")
                           for h in range(HW_)]
                    nmm = KP + lo_kp
                    i = 0
                    for kp in range(KP):
                        for h in range(HW_):
                            nc.tensor.matmul(
                                pss[h][:],
                                x_hi[:, kp, :, g * P:(g + 1) * P],
                                w_sb[:, kp, :, h * NW:(h + 1) * NW],
                                start=(i == 0), stop=(i == nmm - 1),
                                perf_mode=DR)
                        i += 1
                    for kp in range(lo_kp):
                        for h in range(HW_):
                            nc.tensor.matmul(
                                pss[h][:],
                                x_lo[:, kp, :, g * P:(g + 1) * P],
                                w_sb[:, kp, :, h * NW:(h + 1) * NW],
                                start=(i == 0), stop=(i == nmm - 1),
                                perf_mode=DR)
                        i += 1
                    if g == 0 and not ones_at_head:
                        # ones-matmuls sit after the first main group so
                        # the PE never stalls on the reduction chain at a
                        # chunk boundary; r is still ready before the
                        # first eviction needs it.
                        r_sb = emit_norm_tail()
                    for h in range(HW_):
                        ev = evp.tile([P, NW],
                                      bf16 if out_bf16 else mybir.dt.float32,
                                      tag="ev")
                        nc.vector.tensor_scalar_mul(
                            ev[:], pss[h][:], r_sb[:, g:g + 1])
                        nc.sync.dma_start(
                            out[m0 + g * P:m0 + (g + 1) * P,
                                h * NW:(h + 1) * NW],
                            ev[:])
    nc.compile()
    return nc


def _host_prep(x, weight, bias, gamma, lo_kp=LO_KP):
    import jax
    import jax.numpy as jnp
    import ml_dtypes

    e4 = ml_dtypes.float8_e4m3
    w32 = np.asarray(weight, np.float32)
    try:
        # CPU jax reproduces the reference's fp32 reduction order bitwise;
        # ~2 weights sit within 1 ulp of thr, so the order matters.
        with jax.default_device(jax.devices("cpu")[0]):
            thr = np.float32(jnp.mean(jnp.abs(jnp.asarray(w32))))
    except Exception:
        thr = np.float32(np.mean(np.abs(w32)))
    wq = (np.sign(w32) * (np.abs(w32) > thr)).astype(np.float32)
    weff = wq * np.asarray(gamma, np.float32)[None, :]  # [DOUT, DIN]
    # w8[p, kp, i, n] = weff.T[kp*256 + i*128 + p, n]
    # (exact in e4m3 for ternary weights with gamma == 1)
    w8 = np.ascontiguousarray(
        weff.T.reshape(KP, 2, P, DOUT).transpose(2, 0, 1, 3)
    ).astype(e4)  # [P, KP, 2, DOUT]

    x32 = np.asarray(x, np.float32).reshape(M, DIN)
    hi = x32.astype(e4)
    lo = (x32 - hi.astype(np.float32)).astype(e4)

    def pack(a, kp_lim):
        # a[m, f]; f = kp*256 + i*128 + p -> [nch*P, kp, i, MCH]
        return np.ascontiguousarray(
            a.reshape(M // MCH, MCH, KP, 2, P)[:, :, :kp_lim]
            .transpose(0, 4, 2, 3, 1)
        ).reshape((M // MCH) * P, kp_lim, 2, MCH)

    b32 = np.ascontiguousarray(np.asarray(bias, np.float32))
    return pack(hi, KP), (pack(lo, lo_kp) if lo_kp else None), w8, b32


def _in_maps(xhi, xlo, w8):
    maps = []
    for c in range(NCORES):
        m = {"xhi": xhi,
             "wt": np.ascontiguousarray(
                 w8[:, :, :, c * NSHARD:(c + 1) * NSHARD])}
        if xlo is not None:
            m["xlo"] = xlo
        maps.append(m)
    return maps


def kernel(x, weight, bias, gamma):
    from concourse.bass_utils import run_bass_kernel_spmd

    if "nc" not in _CACHE:
        _CACHE["nc"] = build_nc()
    nc = _CACHE["nc"]

    xhi, xlo, w8, b32 = _host_prep(x, weight, bias, gamma)
    in_maps = _in_maps(xhi, xlo, w8)
    res = run_bass_kernel_spmd(nc, in_maps, core_ids=list(range(NCORES)))
    shards = [res.results[c]["out"] for c in range(NCORES)]
    full = np.concatenate(shards, axis=1)
    if np.any(b32):
        full += b32[None, :]
    return np.ascontiguousarray(
        full.reshape(B, S, DOUT).astype(np.float32, copy=False))
